# revision 32
# baseline (speedup 1.0000x reference)
"""Segment-mean + projection kernel for Trainium2 (8 NeuronCores, SPMD).

logits[b] = (mean of x rows in bag b) @ rel_weight.T + bias

Strategy: data-parallel over bags, two precision streams per core.

fp8 stream (bags with >= CSTAR rows, ~92% of rows): rows pre-scaled by
1/count on host and cast to e4m3. Groups of 768 rows (6 tiles of 128);
per pair of tiles the DVE/Pool builds a one-hot A [128, 2, 128]
(A[p,i,m] = seg_i[p]==m) and the PE runs fp8 DoubleRow matmuls (two
128-row contractions per instruction at 0.5 cy/col) accumulating
sums == means into PSUM [128, 512]+[128, 178] fp32.

fp16 stream (bags with < CSTAR rows): quantizing a near-singleton bag to
fp8 fails the tolerance, so these bags bypass matmul entirely. Bags are
grouped by count c in {1..4}; 128 bags form a tile with c row-planes
[128, c, 690] fp16. c=1 planes ARE the means (DMA'd straight into the
means buffer); c>1 reduced with c-1 vector adds.

Shared tail per 128-bag unit: means [128, 690] fp16 -> 6 PE transposes
into one PSUM tile -> one strided copy into a [128, 6, 256] staging pair
-> per 2 units a 6-matmul fp16 projection against W.T chunks -> bias via
ACT -> logitsT [53, 256] -> DRAM. The host compacts valid columns and
adds together the partial columns of bags split across group boundaries
(sums are linear; duplicate bias subtracted), so no fixup pass exists
on-chip. Transposes are software-pipelined one unit behind the segsum
and projections two behind, keeping the PE free of head-of-line stalls.
All data-dependent structure travels as DMA'd tensors, so one program
serves all 8 cores.
"""
import sys
import re

sys.path.insert(0, "/opt/trn_rl_repo")

import numpy as np
import ml_dtypes

N_CORES = 8
D = 690
C = 53
CSTAR = 5  # bags with count < CSTAR use the fp16 direct stream
TILES_PER_GROUP = 12
ROWS_PER_GROUP = 128 * TILES_PER_GROUP  # 1536
DA = 512  # psum split: 512 fp32 = one full bank
DB = D - DA  # 178
D_CHUNKS = 6  # ceil(690/128), last chunk 50 wide
D_LAST = D - 5 * 128  # 50
F8 = ml_dtypes.float8_e4m3


def _apply_walrus_workarounds():
    """This walrus build allows at most one semaphore wait per instruction
    on several opcodes (Drain, Matmult/LDW). Patch Tile's tail drain to use
    standalone wait_ge instructions, and provide a post-pass that hoists
    excess waits onto InstNoOp instructions."""
    from concourse import tile, mybir

    def _patched_drain_and_barrier(self, tick_clock, wait_clock):
        gc = tick_clock.global_clock
        ticks = [int(s) for s in re.findall(r"\d+", repr(gc))]
        allocated = self.sems.allocated()
        for proc, sem in sorted(allocated.items()):
            t = ticks[proc] if proc < len(ticks) else 0
            if t > 0:
                mult = 16 if "DMA" in sem.name else 1
                self.nc.sync.wait_ge(sem, t * mult)
        self.nc.sync.drain()
        self.nc.all_engine_barrier()
        popped = self.nc._tile_sem_poison_stack.pop()
        assert popped is self._sem_poison
        self.nc.clear_and_free_semaphores(list(allocated.values()))
        self.nc.all_engine_barrier()

    tile.TileContext._drain_and_barrier = _patched_drain_and_barrier

    def split_multi_waits(nc, max_waits=1):
        for f in nc.m.functions:
            for b in f.blocks:
                insts = list(b.instructions)
                new = []
                dirty = False
                for inst in insts:
                    si = inst.sync_info
                    if si is not None and len(si.on_wait) > max_waits:
                        waits = list(si.on_wait)
                        extra, keep = waits[:-max_waits], waits[-max_waits:]
                        for k, w in enumerate(extra):
                            nop = mybir.InstNoOp(
                                name=f"{inst.name}-hw{k}", ins=[], outs=[]
                            )
                            nop.engine = inst.engine
                            nop.sync_info = mybir.SyncInfo(
                                on_wait=[w], on_update=[]
                            )
                            new.append(nop)
                        inst.sync_info = mybir.SyncInfo(
                            on_wait=keep, on_update=list(si.on_update)
                        )
                        dirty = True
                    new.append(inst)
                if dirty:
                    b.instructions = new

    return split_multi_waits


def _preprocess(x, scope, n_cores=N_CORES):
    """Split rows into the fp8 (big-bag) and fp16 (small-bag) streams per
    core and pack all DMA tensors. Returns per-core input dicts plus the
    metadata needed to assemble the output."""
    n_sent = x.shape[0]
    n_bags = scope.shape[0] - 1
    scope = np.asarray(scope, dtype=np.int64)
    counts = np.diff(scope)
    assert counts.min() >= 1
    seg_full = np.repeat(np.arange(n_bags, dtype=np.int64), counts)
    inv_c = (1.0 / counts).astype(np.float32)

    # bag cuts equalizing the fp8-stream (big-bag) rows across cores, since
    # those dominate both DMA and PE time
    big_rows_per_bag = np.where(counts >= CSTAR, counts, 0)
    cum_big = np.concatenate([[0], np.cumsum(big_rows_per_bag)])
    bag_cuts = [0]
    for k in range(1, n_cores):
        t = (k * cum_big[-1]) // n_cores
        bag_cuts.append(int(np.searchsorted(cum_big, t)))
    bag_cuts.append(n_bags)

    per_core = []
    for c in range(n_cores):
        b0, b1 = bag_cuts[c], bag_cuts[c + 1]
        big = [b for b in range(b0, b1) if counts[b] >= CSTAR]
        small = {
            k: [b for b in range(b0, b1) if counts[b] == k]
            for k in range(1, CSTAR)
        }
        n_big_rows = int(sum(counts[b] for b in big))
        per_core.append((b0, b1, big, small, n_big_rows))

    G8 = max(
        int(np.ceil(pc[4] / ROWS_PER_GROUP)) for pc in per_core
    )
    NT = {
        k: max(
            int(np.ceil(len(pc[3][k]) / 128)) for pc in per_core
        )
        for k in range(1, CSTAR)
    }
    n_units = G8 + sum(NT.values())
    n_pairs = (n_units + 1) // 2

    cores = []
    for c in range(n_cores):
        b0, b1, big, small, n_big_rows = per_core[c]
        big = np.asarray(big, dtype=np.int64)

        # ---- fp8 stream ----
        R = G8 * ROWS_PER_GROUP
        # big-bag rows in bag order; scale by 1/count
        if len(big):
            row_idx = np.concatenate(
                [np.arange(scope[b], scope[b + 1]) for b in big]
            )
        else:
            row_idx = np.zeros(0, dtype=np.int64)
        xb = np.zeros((R, D), dtype=F8)
        xb[: len(row_idx)] = (
            x[row_idx] * inv_c[seg_full[row_idx]][:, None]
        ).astype(F8)
        # [G*RPG, D] -> [G*128, TPG, D] partition-major within each group
        x8 = np.ascontiguousarray(
            xb.reshape(G8, TILES_PER_GROUP, 128, D).transpose(0, 2, 1, 3)
        ).reshape(G8 * 128, TILES_PER_GROUP, D)

        # ordinal (position within `big`) of each big row
        ord_of_row = np.repeat(
            np.arange(len(big), dtype=np.int64), counts[big]
        )
        # group base ordinal + local seg + assemble ranges
        seg8 = np.full((128, G8 * TILES_PER_GROUP), 128.0, dtype=np.float32)
        # (cast to fp16 below; values 0..128 are exact)
        g_base = np.zeros(G8, dtype=np.int64)
        g_nb = np.zeros(G8, dtype=np.int64)  # bags present in group
        for g in range(G8):
            r0, r1 = g * ROWS_PER_GROUP, min((g + 1) * ROWS_PER_GROUP, len(row_idx))
            if r0 >= len(row_idx):
                g_nb[g] = 0
                continue
            o = ord_of_row[r0:r1]
            g_base[g] = o[0]
            g_nb[g] = o[-1] - o[0] + 1
            assert g_nb[g] <= 128, f"group bag overflow: {g_nb[g]}"
            loc = (o - o[0]).astype(np.float32)
            pad = np.full(ROWS_PER_GROUP - len(loc), 128.0, dtype=np.float32)
            lg = np.concatenate([loc, pad]).reshape(TILES_PER_GROUP, 128)
            seg8[:, g * TILES_PER_GROUP : (g + 1) * TILES_PER_GROUP] = lg.T

        # ---- fp16 stream ----
        x16 = {}
        for k in range(1, CSTAR):
            bags_k = np.asarray(small[k], dtype=np.int64)
            nt = NT[k]
            arr = np.zeros((nt * 128, k, D), dtype=np.float16)
            if len(bags_k):
                rows = (
                    scope[bags_k][:, None] + np.arange(k)[None, :]
                ).reshape(-1)
                vals = (x[rows] * np.float32(1.0 / k)).astype(np.float16)
                arr[: len(bags_k)] = vals.reshape(len(bags_k), k, D)
            x16[k] = arr

        cores.append(
            dict(
                x8=x8,
                seg8=seg8.astype(np.float16),
                x16=x16,
                big=big,
                small={k: np.asarray(v, dtype=np.int64) for k, v in small.items()},
                g_base=g_base,
                g_nb=g_nb,
            )
        )
    return cores, G8, NT, n_units, n_pairs


def _unit_schedule(G8, NT):
    """fp16 tiles interleaved among fp8 groups so their (slow Pool adds +
    DMA) latency hides behind fp8 work. Must be identical between program
    build and output assembly."""
    s16_units = []
    for k in range(1, CSTAR):
        s16_units += [("s16", k, t) for t in range(NT[k])]
    units = []
    stride = max(1, G8 // (len(s16_units) + 1))
    si = 0
    for g in range(G8):
        units.append(("g8", g))
        if (g + 1) % stride == 0 and si < len(s16_units):
            units.append(s16_units[si])
            si += 1
    units += s16_units[si:]
    return units


def _build_program(G8, NT, n_units, n_pairs):
    import concourse.bass as bass
    import concourse.mybir as mybir
    from concourse import tile

    dt = mybir.dt
    DR = mybir.MatmulPerfMode.DoubleRow
    nc = bass.Bass()

    x8_d = nc.declare_dram_parameter(
        "x8", [G8 * 128, TILES_PER_GROUP, D], dt.float8e4, isOutput=False
    )
    seg_d = nc.declare_dram_parameter(
        "seg8", [128, G8 * TILES_PER_GROUP], dt.float16, isOutput=False
    )
    x16_d = {}
    for k in range(1, CSTAR):
        if NT[k] == 0:
            continue
        if k == 1:
            x16_d[k] = nc.declare_dram_parameter(
                "x16_1", [NT[1] * 128, D], dt.float16, isOutput=False
            )
        else:
            x16_d[k] = nc.declare_dram_parameter(
                f"x16_{k}", [NT[k] * 128, k, D], dt.float16, isOutput=False
            )
    iota_d = nc.declare_dram_parameter("iota", [128, TILES_PER_GROUP * 128], dt.float16, isOutput=False)
    ident_d = nc.declare_dram_parameter("ident", [128, 128], dt.float16, isOutput=False)
    wt_d = nc.declare_dram_parameter("wt", [128, 768], dt.float16, isOutput=False)
    bias_d = nc.declare_dram_parameter("bias", [C, 1], dt.float32, isOutput=False)
    out_d = nc.declare_dram_parameter(
        "out", [C, n_pairs * 256], dt.float32, isOutput=True
    )

    units = _unit_schedule(G8, NT)
    assert len(units) == n_units

    with tile.TileContext(nc) as tc:
        with (
            tc.tile_pool(name="const", bufs=1) as cpool,
            tc.tile_pool(name="x8in", bufs=6) as x8pool,
            tc.tile_pool(name="x16in", bufs=4) as x16pool,
            tc.tile_pool(name="onehot", bufs=4) as apool,
            tc.tile_pool(name="means", bufs=4) as mpool,
            tc.tile_pool(name="s16means", bufs=1) as s16pool,
            tc.tile_pool(name="mgt", bufs=2) as tpool,
            tc.tile_pool(name="outs", bufs=2) as opool,
            tc.tile_pool(name="ps_sum", bufs=3, space="PSUM") as pspool,
            tc.tile_pool(name="ps_tr", bufs=1, space="PSUM") as ptpool,
            tc.tile_pool(name="ps_proj", bufs=1, space="PSUM") as pppool,
        ):
            iota_t = cpool.tile([128, TILES_PER_GROUP, 128], dt.float16)
            ident_t = cpool.tile([128, 128], dt.float16)
            seg_t = cpool.tile([128, G8 * TILES_PER_GROUP], dt.float16)
            wt_t = cpool.tile([128, 768], dt.float16)
            bias_t = cpool.tile([C, 1], dt.float32)

            nc.sync.dma_start(out=iota_t[:], in_=iota_d[:])
            nc.sync.dma_start(out=ident_t[:], in_=ident_d[:])
            nc.sync.dma_start(out=seg_t[:], in_=seg_d[:])
            nc.sync.dma_start(out=wt_t[:], in_=wt_d[:])
            nc.sync.dma_start(out=bias_t[:], in_=bias_d[:])

            means = [None] * n_units  # SBUF means tiles
            pst = [None] * n_units  # transpose PSUM tiles
            a_tiles = {}
            mgt = None
            tcopy_step = {}  # pair q -> step its last tcopy was emitted
            proj_done = set()

            # ---- all fp16-stream means produced up front: their DMAs fill
            # the initially-idle DMA rings and the slow Pool adds finish
            # long before their transposes come due ----
            for u0, kind in enumerate(units):
                if kind[0] != "s16":
                    continue
                _, k, t = kind
                m_t = s16pool.tile([128, D], dt.float16, tag=f"sm{u0}")
                means[u0] = m_t
                if k == 1:
                    nc.gpsimd.dma_start(
                        out=m_t[:, :],
                        in_=x16_d[1][t * 128 : (t + 1) * 128, :],
                    )
                else:
                    x_t = x16pool.tile([128, k, D], dt.float16, tag=f"x16_{k}")
                    nc.gpsimd.dma_start(
                        out=x_t[:, :, :],
                        in_=x16_d[k][t * 128 : (t + 1) * 128, :, :],
                    )
                    nc.gpsimd.tensor_tensor(
                        out=m_t[:],
                        in0=x_t[:, 0, :],
                        in1=x_t[:, 1, :],
                        op=mybir.AluOpType.add,
                    )
                    for j in range(2, k):
                        nc.gpsimd.tensor_tensor(
                            out=m_t[:],
                            in0=m_t[:],
                            in1=x_t[:, j, :],
                            op=mybir.AluOpType.add,
                        )

            def build_onehot(uu):
                # one-hot A[p,t,m] = (iota[p,m]==seg[p,t]) for unit uu,
                # built one step ahead so the PE never waits on it.
                # Half on DVE, half on Pool.
                if units[uu][0] != "g8":
                    return
                g = units[uu][1]
                a_t = apool.tile(
                    [128, TILES_PER_GROUP, 128], dt.float8e4, tag="a"
                )
                a_tiles[uu] = a_t
                c0 = g * TILES_PER_GROUP
                HT = TILES_PER_GROUP // 2
                for hh, eng in ((0, nc.vector), (1, nc.vector)):
                    eng.tensor_tensor(
                        out=a_t[:, hh * HT : (hh + 1) * HT, :],
                        in0=iota_t[:, hh * HT : (hh + 1) * HT, :],
                        in1=seg_t[
                            :, c0 + hh * HT : c0 + (hh + 1) * HT, None
                        ].broadcast_to([128, HT, 128]),
                        op=mybir.AluOpType.is_equal,
                    )

            n_steps = n_units + 3
            for u in range(n_steps):
                # ---- B: transposes + staging copy for unit u-1 ----
                v = u - 1
                if 0 <= v < n_units:
                    ps_t = ptpool.tile([128, D_CHUNKS, 128], dt.float16, tag="pt")
                    pst[v] = ps_t
                    m_t = means[v]
                    for dch in range(D_CHUNKS):
                        w = 128 if dch < 5 else D_LAST
                        nc.tensor.transpose(
                            ps_t[0:w, dch, :],
                            m_t[:, dch * 128 : dch * 128 + w],
                            ident_t[:],
                        )
                    h = v % 2
                    if h == 0:
                        mgt = tpool.tile([128, D_CHUNKS, 256], dt.float16, tag="mgt")
                    nc.vector.tensor_copy(
                        mgt[:, 0:D_CHUNKS, h * 128 : h * 128 + 128],
                        ps_t[:, 0:D_CHUNKS, :],
                    )
                    q = v // 2
                    if h == 1 or v == n_units - 1:
                        tcopy_step[q] = (u, mgt)
                    means[v] = None
                    pst[v] = None

                # ---- C: projection for any pair fully staged before this step ----
                for q, (step, mg) in list(tcopy_step.items()):
                    if step < u and q not in proj_done:
                        proj_done.add(q)
                        pp = pppool.tile([128, 256], dt.float32, tag="pp")
                        for dch in range(D_CHUNKS):
                            w = 128 if dch < 5 else D_LAST
                            nc.tensor.matmul(
                                pp[:],
                                wt_t[0:w, dch * 128 : (dch + 1) * 128],
                                mg[0:w, dch, 0:256],
                                start=(dch == 0),
                                stop=(dch == D_CHUNKS - 1),
                            )
                        out_sb = opool.tile([C, 256], dt.float32, tag="o")
                        nc.scalar.activation(
                            out_sb[:],
                            pp[0:C, :],
                            mybir.ActivationFunctionType.Identity,
                            bias=bias_t[:],
                        )
                        nc.scalar.dma_start(
                            out=out_d[:, q * 256 : (q + 1) * 256], in_=out_sb[:]
                        )
                        del tcopy_step[q]
                if u == 0 and n_units:
                    build_onehot(0)
                if u + 1 < n_units:
                    build_onehot(u + 1)
                # ---- A: produce means for unit u (g8 only; s16 premade) ----
                if u < n_units and units[u][0] == "g8":
                    kind = units[u]
                    if True:
                        m_t = mpool.tile([128, D], dt.float16, tag="m")
                        means[u] = m_t
                        g = kind[1]
                        x_t = x8pool.tile(
                            [128, TILES_PER_GROUP, D], dt.float8e4, tag="x8"
                        )
                        nc.sync.dma_start(
                            out=x_t[:, :, :],
                            in_=x8_d[g * 128 : (g + 1) * 128, :, :],
                        )
                        a_t = a_tiles.pop(u)
                        ps = pspool.tile([128, D], dt.float32, tag="ps")
                        for p in range(TILES_PER_GROUP // 2):
                            nc.tensor.matmul(
                                ps[:, 0:DA],
                                a_t[:, 2 * p : 2 * p + 2, :],
                                x_t[:, 2 * p : 2 * p + 2, 0:DA],
                                start=(p == 0),
                                stop=(p == TILES_PER_GROUP // 2 - 1),
                                perf_mode=DR,
                            )
                            nc.tensor.matmul(
                                ps[:, DA:D],
                                a_t[:, 2 * p : 2 * p + 2, :],
                                x_t[:, 2 * p : 2 * p + 2, DA:D],
                                start=(p == 0),
                                stop=(p == TILES_PER_GROUP // 2 - 1),
                                perf_mode=DR,
                            )
                        nc.scalar.copy(m_t[:, 0:D], ps[:, 0:D])

    return nc


def prepare(x, scope, rel_weight, bias):
    """Build the SPMD program + per-core input maps. Returns a dict with
    everything needed to execute and assemble the output."""
    split_multi_waits = _apply_walrus_workarounds()

    x = np.asarray(x, dtype=np.float32)
    scope_np = np.asarray(scope)
    rel_weight = np.asarray(rel_weight, dtype=np.float32)
    bias = np.asarray(bias, dtype=np.float32)
    n_bags = scope_np.shape[0] - 1

    cores, G8, NT, n_units, n_pairs = _preprocess(x, scope_np)
    nc = _build_program(G8, NT, n_units, n_pairs)
    split_multi_waits(nc)

    iota = np.tile(np.arange(128, dtype=np.float16), (128, TILES_PER_GROUP))
    ident = np.eye(128, dtype=np.float16)
    wt = np.zeros((128, 768), dtype=np.float16)
    wpad = np.zeros((C, 768), dtype=np.float32)
    wpad[:, :D] = rel_weight
    for d in range(6):
        wt[:, d * 128 : d * 128 + C] = wpad[:, d * 128 : (d + 1) * 128].T
    bias_in = bias.reshape(C, 1).copy()

    in_maps = []
    for c in range(N_CORES):
        cd = cores[c]
        im = {
            "x8": cd["x8"],
            "seg8": cd["seg8"],
            "iota": iota,
            "ident": ident,
            "wt": wt,
            "bias": bias_in,
        }
        for k in range(1, CSTAR):
            if NT[k] == 0:
                continue
            arr = cd["x16"][k]
            im[f"x16_{k}"] = arr[:, 0, :].copy() if k == 1 else arr
        in_maps.append(im)

    units = _unit_schedule(G8, NT)

    def assemble(results):
        logits_t = np.zeros((C, n_bags), dtype=np.float32)
        nadd = np.zeros(n_bags, dtype=np.int64)
        for c in range(N_CORES):
            out = results[c]["out"]  # [C, n_pairs*256]
            cd = cores[c]

            def unit_cols(u):
                qq, hh = u // 2, u % 2
                c0 = qq * 256 + hh * 128
                return out[:, c0 : c0 + 128]

            big = cd["big"]
            for u, unit in enumerate(units):
                if unit[0] == "g8":
                    g = unit[1]
                    nb = int(cd["g_nb"][g])
                    if nb == 0:
                        continue
                    o0 = int(cd["g_base"][g])
                    bags = big[o0 : o0 + nb]
                    cols = unit_cols(u)
                    logits_t[:, bags] += cols[:, 0:nb]
                    nadd[bags] += 1
                else:
                    _, k, t = unit
                    sel = cd["small"][k][t * 128 : (t + 1) * 128]
                    if len(sel):
                        cols = unit_cols(u)
                        logits_t[:, sel] = cols[:, 0 : len(sel)]
                        nadd[sel] += 1
        # bags summed across multiple groups got bias multiple times
        extra = (nadd - 1).astype(np.float32)
        logits_t -= bias_in * extra[None, :]
        return np.ascontiguousarray(logits_t.T)

    return dict(
        nc=nc, in_maps=in_maps, assemble=assemble, G8=G8, NT=NT, n_pairs=n_pairs
    )


def kernel(x, scope, rel_weight, bias):
    from concourse.bass_utils import run_bass_kernel_spmd

    p = prepare(x, scope, rel_weight, bias)
    res = run_bass_kernel_spmd(p["nc"], p["in_maps"], list(range(N_CORES)))
    return p["assemble"](res.results)


# revision 38
# speedup vs baseline: 1.0679x; 1.0679x over previous
"""Segment-mean + projection kernel for Trainium2 (8 NeuronCores, SPMD).

logits[b] = (mean of x rows in bag b) @ rel_weight.T + bias

Strategy: data-parallel over bags, two precision streams per core.

fp8 stream (bags with >= CSTAR rows, ~92% of rows): rows pre-scaled by
1/count on host and cast to e4m3. Groups of 768 rows (6 tiles of 128);
per pair of tiles the DVE/Pool builds a one-hot A [128, 2, 128]
(A[p,i,m] = seg_i[p]==m) and the PE runs fp8 DoubleRow matmuls (two
128-row contractions per instruction at 0.5 cy/col) accumulating
sums == means into PSUM [128, 512]+[128, 178] fp32.

fp16 stream (bags with < CSTAR rows): quantizing a near-singleton bag to
fp8 fails the tolerance, so these bags bypass matmul entirely. Bags are
grouped by count c in {1..4}; 128 bags form a tile with c row-planes
[128, c, 690] fp16. c=1 planes ARE the means (DMA'd straight into the
means buffer); c>1 reduced with c-1 vector adds.

Shared tail per 128-bag unit: means [128, 690] fp16 -> 6 PE transposes
into one PSUM tile -> one strided copy into a [128, 6, 256] staging pair
-> per 2 units a 6-matmul fp16 projection against W.T chunks -> bias via
ACT -> logitsT [53, 256] -> DRAM. The host compacts valid columns and
adds together the partial columns of bags split across group boundaries
(sums are linear; duplicate bias subtracted), so no fixup pass exists
on-chip. Transposes are software-pipelined one unit behind the segsum
and projections two behind, keeping the PE free of head-of-line stalls.
All data-dependent structure travels as DMA'd tensors, so one program
serves all 8 cores.
"""
import sys
import re

sys.path.insert(0, "/opt/trn_rl_repo")

import numpy as np
import ml_dtypes

N_CORES = 8
D = 690
C = 53
CSTAR = 5  # bags with count < CSTAR use the fp16 direct stream
TILES_PER_GROUP = 12
ROWS_PER_GROUP = 128 * TILES_PER_GROUP  # 1536
DA = 512  # psum split: 512 fp32 = one full bank
DB = D - DA  # 178
D_CHUNKS = 6  # ceil(690/128), last chunk 50 wide
D_LAST = D - 5 * 128  # 50
F8 = ml_dtypes.float8_e4m3


def _apply_walrus_workarounds():
    """This walrus build allows at most one semaphore wait per instruction
    on several opcodes (Drain, Matmult/LDW). Patch Tile's tail drain to use
    standalone wait_ge instructions, and provide a post-pass that hoists
    excess waits onto InstNoOp instructions."""
    from concourse import tile, mybir

    def _patched_drain_and_barrier(self, tick_clock, wait_clock):
        gc = tick_clock.global_clock
        ticks = [int(s) for s in re.findall(r"\d+", repr(gc))]
        allocated = self.sems.allocated()
        for proc, sem in sorted(allocated.items()):
            t = ticks[proc] if proc < len(ticks) else 0
            if t > 0:
                mult = 16 if "DMA" in sem.name else 1
                self.nc.sync.wait_ge(sem, t * mult)
        self.nc.sync.drain()
        self.nc.all_engine_barrier()
        popped = self.nc._tile_sem_poison_stack.pop()
        assert popped is self._sem_poison
        self.nc.clear_and_free_semaphores(list(allocated.values()))
        self.nc.all_engine_barrier()

    tile.TileContext._drain_and_barrier = _patched_drain_and_barrier

    def split_multi_waits(nc, max_waits=1):
        for f in nc.m.functions:
            for b in f.blocks:
                insts = list(b.instructions)
                new = []
                dirty = False
                for inst in insts:
                    si = inst.sync_info
                    if si is not None and len(si.on_wait) > max_waits:
                        waits = list(si.on_wait)
                        extra, keep = waits[:-max_waits], waits[-max_waits:]
                        for k, w in enumerate(extra):
                            nop = mybir.InstNoOp(
                                name=f"{inst.name}-hw{k}", ins=[], outs=[]
                            )
                            nop.engine = inst.engine
                            nop.sync_info = mybir.SyncInfo(
                                on_wait=[w], on_update=[]
                            )
                            new.append(nop)
                        inst.sync_info = mybir.SyncInfo(
                            on_wait=keep, on_update=list(si.on_update)
                        )
                        dirty = True
                    new.append(inst)
                if dirty:
                    b.instructions = new

    return split_multi_waits


def _preprocess(x, scope, n_cores=N_CORES):
    """Split rows into the fp8 (big-bag) and fp16 (small-bag) streams per
    core and pack all DMA tensors. Returns per-core input dicts plus the
    metadata needed to assemble the output."""
    n_sent = x.shape[0]
    n_bags = scope.shape[0] - 1
    scope = np.asarray(scope, dtype=np.int64)
    counts = np.diff(scope)
    assert counts.min() >= 1
    seg_full = np.repeat(np.arange(n_bags, dtype=np.int64), counts)
    inv_c = (1.0 / counts).astype(np.float32)

    # bag cuts equalizing the fp8-stream (big-bag) rows across cores, since
    # those dominate both DMA and PE time
    big_rows_per_bag = np.where(counts >= CSTAR, counts, 0)
    cum_big = np.concatenate([[0], np.cumsum(big_rows_per_bag)])
    bag_cuts = [0]
    for k in range(1, n_cores):
        t = (k * cum_big[-1]) // n_cores
        bag_cuts.append(int(np.searchsorted(cum_big, t)))
    bag_cuts.append(n_bags)

    per_core = []
    for c in range(n_cores):
        b0, b1 = bag_cuts[c], bag_cuts[c + 1]
        big = [b for b in range(b0, b1) if counts[b] >= CSTAR]
        small = {
            k: [b for b in range(b0, b1) if counts[b] == k]
            for k in range(1, CSTAR)
        }
        n_big_rows = int(sum(counts[b] for b in big))
        per_core.append((b0, b1, big, small, n_big_rows))

    G8 = max(
        int(np.ceil(pc[4] / ROWS_PER_GROUP)) for pc in per_core
    )
    NT = {
        k: max(
            int(np.ceil(len(pc[3][k]) / 128)) for pc in per_core
        )
        for k in range(1, CSTAR)
    }
    n_units = G8 + sum(NT.values())
    n_pairs = (n_units + 1) // 2

    cores = []
    for c in range(n_cores):
        b0, b1, big, small, n_big_rows = per_core[c]
        big = np.asarray(big, dtype=np.int64)

        # ---- fp8 stream ----
        R = G8 * ROWS_PER_GROUP
        # big-bag rows in bag order; scale by 1/count
        if len(big):
            row_idx = np.concatenate(
                [np.arange(scope[b], scope[b + 1]) for b in big]
            )
        else:
            row_idx = np.zeros(0, dtype=np.int64)
        xb = np.zeros((R, D), dtype=F8)
        xb[: len(row_idx)] = (
            x[row_idx] * inv_c[seg_full[row_idx]][:, None]
        ).astype(F8)
        # [G*RPG, D] -> [G*128, TPG, D] partition-major per group
        x8 = np.ascontiguousarray(
            xb.reshape(G8, TILES_PER_GROUP, 128, D).transpose(0, 2, 1, 3)
        ).reshape(G8 * 128, TILES_PER_GROUP, D)

        # ordinal (position within `big`) of each big row
        ord_of_row = np.repeat(
            np.arange(len(big), dtype=np.int64), counts[big]
        )
        # group base ordinal + local seg + assemble ranges
        seg8 = np.full((128, G8 * TILES_PER_GROUP), 128.0, dtype=np.float32)
        # (cast to fp16 below; values 0..128 are exact)
        g_base = np.zeros(G8, dtype=np.int64)
        g_nb = np.zeros(G8, dtype=np.int64)  # bags present in group
        for g in range(G8):
            r0, r1 = g * ROWS_PER_GROUP, min((g + 1) * ROWS_PER_GROUP, len(row_idx))
            if r0 >= len(row_idx):
                g_nb[g] = 0
                continue
            o = ord_of_row[r0:r1]
            g_base[g] = o[0]
            g_nb[g] = o[-1] - o[0] + 1
            assert g_nb[g] <= 128, f"group bag overflow: {g_nb[g]}"
            loc = (o - o[0]).astype(np.float32)
            pad = np.full(ROWS_PER_GROUP - len(loc), 128.0, dtype=np.float32)
            lg = np.concatenate([loc, pad]).reshape(TILES_PER_GROUP, 128)
            seg8[:, g * TILES_PER_GROUP : (g + 1) * TILES_PER_GROUP] = lg.T

        # host-built one-hot: a8[p, t, m] = (seg8[p, g*TPG+t] == m)
        segv = seg8.T.reshape(G8, TILES_PER_GROUP, 128).transpose(0, 2, 1)
        a8 = (
            segv[:, :, :, None] == np.arange(128, dtype=np.float32)
        ).astype(F8).reshape(G8 * 128, TILES_PER_GROUP, 128)

        # ---- fp16 stream ----
        x16 = {}
        for k in range(1, CSTAR):
            bags_k = np.asarray(small[k], dtype=np.int64)
            nt = NT[k]
            arr = np.zeros((nt * 128, k, D), dtype=np.float16)
            if len(bags_k):
                rows = (
                    scope[bags_k][:, None] + np.arange(k)[None, :]
                ).reshape(-1)
                vals = (x[rows] * np.float32(1.0 / k)).astype(np.float16)
                arr[: len(bags_k)] = vals.reshape(len(bags_k), k, D)
            x16[k] = arr

        cores.append(
            dict(
                x8=x8,
                a8=a8,
                seg8=seg8.astype(np.float16),
                x16=x16,
                big=big,
                small={k: np.asarray(v, dtype=np.int64) for k, v in small.items()},
                g_base=g_base,
                g_nb=g_nb,
            )
        )
    return cores, G8, NT, n_units, n_pairs


def _unit_schedule(G8, NT):
    """fp16 tiles interleaved among fp8 groups so their (slow Pool adds +
    DMA) latency hides behind fp8 work. Must be identical between program
    build and output assembly."""
    s16_units = []
    for k in range(1, CSTAR):
        s16_units += [("s16", k, t) for t in range(NT[k])]
    units = []
    stride = max(1, G8 // (len(s16_units) + 1))
    si = 0
    for g in range(G8):
        units.append(("g8", g))
        if (g + 1) % stride == 0 and si < len(s16_units):
            units.append(s16_units[si])
            si += 1
    units += s16_units[si:]
    return units


def _build_program(G8, NT, n_units, n_pairs):
    import concourse.bass as bass
    import concourse.mybir as mybir
    from concourse import tile

    dt = mybir.dt
    DR = mybir.MatmulPerfMode.DoubleRow
    nc = bass.Bass()

    x8_d = nc.declare_dram_parameter(
        "x8", [G8 * 128, TILES_PER_GROUP, D], dt.float8e4, isOutput=False
    )
    a8_d = nc.declare_dram_parameter(
        "a8", [G8 * 128, TILES_PER_GROUP, 128], dt.float8e4, isOutput=False
    )
    x16_d = {}
    for k in range(1, CSTAR):
        if NT[k] == 0:
            continue
        if k == 1:
            x16_d[k] = nc.declare_dram_parameter(
                "x16_1", [NT[1] * 128, D], dt.float16, isOutput=False
            )
        else:
            x16_d[k] = nc.declare_dram_parameter(
                f"x16_{k}", [NT[k] * 128, k, D], dt.float16, isOutput=False
            )
    ident_d = nc.declare_dram_parameter("ident", [128, 128], dt.float16, isOutput=False)
    wt_d = nc.declare_dram_parameter("wt", [128, 768], dt.float16, isOutput=False)
    bias_d = nc.declare_dram_parameter("bias", [C, 1], dt.float32, isOutput=False)
    out_d = nc.declare_dram_parameter(
        "out", [C, n_pairs * 256], dt.float32, isOutput=True
    )

    units = _unit_schedule(G8, NT)
    assert len(units) == n_units

    with tile.TileContext(nc) as tc:
        with (
            tc.tile_pool(name="const", bufs=1) as cpool,
            tc.tile_pool(name="x8in", bufs=6) as x8pool,
            tc.tile_pool(name="x16in", bufs=4) as x16pool,
            tc.tile_pool(name="means", bufs=4) as mpool,
            tc.tile_pool(name="s16means", bufs=1) as s16pool,
            tc.tile_pool(name="mgt", bufs=2) as tpool,
            tc.tile_pool(name="outs", bufs=2) as opool,
            tc.tile_pool(name="ps_sum", bufs=3, space="PSUM") as pspool,
            tc.tile_pool(name="ps_tr", bufs=1, space="PSUM") as ptpool,
            tc.tile_pool(name="ps_proj", bufs=1, space="PSUM") as pppool,
        ):
            ident_t = cpool.tile([128, 128], dt.float16)
            wt_t = cpool.tile([128, 768], dt.float16)
            bias_t = cpool.tile([C, 1], dt.float32)

            nc.sync.dma_start(out=ident_t[:], in_=ident_d[:])
            nc.sync.dma_start(out=wt_t[:], in_=wt_d[:])
            nc.sync.dma_start(out=bias_t[:], in_=bias_d[:])

            means = [None] * n_units  # SBUF means tiles
            pst = [None] * n_units  # transpose PSUM tiles
            mgt = None
            tcopy_step = {}  # pair q -> step its last tcopy was emitted
            proj_done = set()

            n_steps = n_units + 3
            for u in range(n_steps):
                # ---- A: produce means for unit u ----
                if u < n_units:
                    kind = units[u]
                    m_t = mpool.tile([128, D], dt.float16, tag="m")
                    means[u] = m_t
                    if kind[0] == "g8":
                        g = kind[1]
                        x_t = x8pool.tile(
                            [128, TILES_PER_GROUP, D], dt.float8e4, tag="x8"
                        )
                        nc.sync.dma_start(
                            out=x_t[:, :, :],
                            in_=x8_d[g * 128 : (g + 1) * 128, :, :],
                        )
                        a_t = x8pool.tile(
                            [128, TILES_PER_GROUP, 128], dt.float8e4, tag="a8"
                        )
                        nc.sync.dma_start(
                            out=a_t[:, :, :],
                            in_=a8_d[g * 128 : (g + 1) * 128, :, :],
                        )
                        ps = pspool.tile([128, D], dt.float32, tag="ps")
                        for p in range(TILES_PER_GROUP // 2):
                            nc.tensor.matmul(
                                ps[:, 0:DA],
                                a_t[:, 2 * p : 2 * p + 2, :],
                                x_t[:, 2 * p : 2 * p + 2, 0:DA],
                                start=(p == 0),
                                stop=(p == TILES_PER_GROUP // 2 - 1),
                                perf_mode=DR,
                            )
                            nc.tensor.matmul(
                                ps[:, DA:D],
                                a_t[:, 2 * p : 2 * p + 2, :],
                                x_t[:, 2 * p : 2 * p + 2, DA:D],
                                start=(p == 0),
                                stop=(p == TILES_PER_GROUP // 2 - 1),
                                perf_mode=DR,
                            )
                        nc.scalar.copy(m_t[:, 0:D], ps[:, 0:D])
                    else:
                        _, k, t = kind
                        if k == 1:
                            nc.sync.dma_start(
                                out=m_t[:, :],
                                in_=x16_d[1][t * 128 : (t + 1) * 128, :],
                            )
                        else:
                            x_t = x16pool.tile(
                                [128, k, D], dt.float16, tag=f"x16_{k}"
                            )
                            nc.sync.dma_start(
                                out=x_t[:, :, :],
                                in_=x16_d[k][t * 128 : (t + 1) * 128, :, :],
                            )
                            nc.gpsimd.tensor_tensor(
                                out=m_t[:],
                                in0=x_t[:, 0, :],
                                in1=x_t[:, 1, :],
                                op=mybir.AluOpType.add,
                            )
                            for j in range(2, k):
                                nc.gpsimd.tensor_tensor(
                                    out=m_t[:],
                                    in0=m_t[:],
                                    in1=x_t[:, j, :],
                                    op=mybir.AluOpType.add,
                                )

                # ---- B: transposes + staging copy for unit u-1 ----
                v = u - 1
                if 0 <= v < n_units:
                    ps_t = ptpool.tile([128, D_CHUNKS, 128], dt.float16, tag="pt")
                    pst[v] = ps_t
                    m_t = means[v]
                    for dch in range(D_CHUNKS):
                        w = 128 if dch < 5 else D_LAST
                        nc.tensor.transpose(
                            ps_t[0:w, dch, :],
                            m_t[:, dch * 128 : dch * 128 + w],
                            ident_t[:],
                        )
                    h = v % 2
                    if h == 0:
                        mgt = tpool.tile([128, D_CHUNKS, 256], dt.float16, tag="mgt")
                    nc.vector.tensor_copy(
                        mgt[:, 0:D_CHUNKS, h * 128 : h * 128 + 128],
                        ps_t[:, 0:D_CHUNKS, :],
                    )
                    q = v // 2
                    if h == 1 or v == n_units - 1:
                        tcopy_step[q] = (u, mgt)
                    means[v] = None
                    pst[v] = None

                # ---- C: projection for any pair fully staged before this step ----
                for q, (step, mg) in list(tcopy_step.items()):
                    if step < u and q not in proj_done:
                        proj_done.add(q)
                        pp = pppool.tile([128, 256], dt.float32, tag="pp")
                        for dch in range(D_CHUNKS):
                            w = 128 if dch < 5 else D_LAST
                            nc.tensor.matmul(
                                pp[:],
                                wt_t[0:w, dch * 128 : (dch + 1) * 128],
                                mg[0:w, dch, 0:256],
                                start=(dch == 0),
                                stop=(dch == D_CHUNKS - 1),
                            )
                        out_sb = opool.tile([C, 256], dt.float32, tag="o")
                        nc.scalar.activation(
                            out_sb[:],
                            pp[0:C, :],
                            mybir.ActivationFunctionType.Identity,
                            bias=bias_t[:],
                        )
                        nc.scalar.dma_start(
                            out=out_d[:, q * 256 : (q + 1) * 256], in_=out_sb[:]
                        )
                        del tcopy_step[q]
    return nc


def prepare(x, scope, rel_weight, bias):
    """Build the SPMD program + per-core input maps. Returns a dict with
    everything needed to execute and assemble the output."""
    split_multi_waits = _apply_walrus_workarounds()

    x = np.asarray(x, dtype=np.float32)
    scope_np = np.asarray(scope)
    rel_weight = np.asarray(rel_weight, dtype=np.float32)
    bias = np.asarray(bias, dtype=np.float32)
    n_bags = scope_np.shape[0] - 1

    cores, G8, NT, n_units, n_pairs = _preprocess(x, scope_np)
    nc = _build_program(G8, NT, n_units, n_pairs)
    split_multi_waits(nc)

    ident = np.eye(128, dtype=np.float16)
    wt = np.zeros((128, 768), dtype=np.float16)
    wpad = np.zeros((C, 768), dtype=np.float32)
    wpad[:, :D] = rel_weight
    for d in range(6):
        wt[:, d * 128 : d * 128 + C] = wpad[:, d * 128 : (d + 1) * 128].T
    bias_in = bias.reshape(C, 1).copy()

    in_maps = []
    for c in range(N_CORES):
        cd = cores[c]
        im = {
            "x8": cd["x8"],
            "a8": cd["a8"],
            "ident": ident,
            "wt": wt,
            "bias": bias_in,
        }
        for k in range(1, CSTAR):
            if NT[k] == 0:
                continue
            arr = cd["x16"][k]
            im[f"x16_{k}"] = arr[:, 0, :].copy() if k == 1 else arr
        in_maps.append(im)

    units = _unit_schedule(G8, NT)

    def assemble(results):
        logits_t = np.zeros((C, n_bags), dtype=np.float32)
        nadd = np.zeros(n_bags, dtype=np.int64)
        for c in range(N_CORES):
            out = results[c]["out"]  # [C, n_pairs*256]
            cd = cores[c]

            def unit_cols(u):
                qq, hh = u // 2, u % 2
                c0 = qq * 256 + hh * 128
                return out[:, c0 : c0 + 128]

            big = cd["big"]
            for u, unit in enumerate(units):
                if unit[0] == "g8":
                    g = unit[1]
                    nb = int(cd["g_nb"][g])
                    if nb == 0:
                        continue
                    o0 = int(cd["g_base"][g])
                    bags = big[o0 : o0 + nb]
                    cols = unit_cols(u)
                    logits_t[:, bags] += cols[:, 0:nb]
                    nadd[bags] += 1
                else:
                    _, k, t = unit
                    sel = cd["small"][k][t * 128 : (t + 1) * 128]
                    if len(sel):
                        cols = unit_cols(u)
                        logits_t[:, sel] = cols[:, 0 : len(sel)]
                        nadd[sel] += 1
        # bags summed across multiple groups got bias multiple times
        extra = (nadd - 1).astype(np.float32)
        logits_t -= bias_in * extra[None, :]
        return np.ascontiguousarray(logits_t.T)

    return dict(
        nc=nc, in_maps=in_maps, assemble=assemble, G8=G8, NT=NT, n_pairs=n_pairs
    )


def kernel(x, scope, rel_weight, bias):
    from concourse.bass_utils import run_bass_kernel_spmd

    p = prepare(x, scope, rel_weight, bias)
    res = run_bass_kernel_spmd(p["nc"], p["in_maps"], list(range(N_CORES)))
    return p["assemble"](res.results)
